# revision 1
# baseline (speedup 1.0000x reference)
"""MoE (top-2, capacity-dropped) Trainium2 kernel, expert-parallel across 8 NeuronCores.

v2 strategy (vs v1 baseline):
  - Host pre-casts weights/features to bf16 and pre-transposes features
    ([F,B] "xT"), split into hi/lo bf16 halves so the router scores match
    fp32 selection exactly (3-term split matmul, fp32 accumulation).
  - Router computes scoresT [E,512-token-chunk] with wide N=512 matmuls
    (no per-tile feature transposes), transposes the tiny [16,128] score
    tiles back, then does the baseline top-2/softmax math per 128-token tile.
  - Capacity/slot building is fully batched: slot ids for all 32K
    (token,expert) pairs are computed in a handful of wide DVE ops and
    written with TWO indirect DMA scatters (v1 used 256), directly into
    gather-ready DRAM layouts (16-wrapped idx table, 128-wrapped weight
    table).
  - Experts run with SBLK=512 (half the matmul count of v1, N=512 per
    PE instruction), weights DMA-loaded in bf16 (no on-chip conversion),
    expert-0 weights prefetched during the router, combine scale applied
    on the otherwise-idle Activation engine.
  - Weighted combine via dma_scatter_add into a [B+128,O] bf16 partial,
    ReduceScatter across 8 cores, each core emits its row-shard fp32.
"""

import sys

for _p in ("/opt/trn_rl_repo", "/opt/pypackages"):
    if _p not in sys.path:
        sys.path.append(_p)

import numpy as np

from concourse import bass, mybir, tile, library_config
from concourse import bacc

FP32 = mybir.dt.float32
BF16 = mybir.dt.bfloat16
I32 = mybir.dt.int32
I16 = mybir.dt.int16


def build_moe(B=16384, F=1024, H=4096, O=1024, E=16, NCORES=8, CAP=2560,
              SBLK=512, CHUNK=512, partial_dtype=BF16, debug_outputs=False,
              n_iters=1, skip_collective=False, skip_dispatch_build=False):
    EL = E // NCORES              # experts per core
    NT = B // 128                 # token tiles
    NCH = B // CHUNK              # router chunks
    TPC = CHUNK // 128            # token tiles per chunk
    FC = F // 128
    HC = H // 128
    OC = O // 128
    NBLK = CAP // SBLK            # slot blocks per expert
    G = SBLK // 128
    S = EL * CAP                  # total slots per core
    SW16 = S // 16
    SW128 = S // 128
    BP = B + 128                  # partial rows (dump row at B)
    RSR = BP // NCORES            # ReduceScatter rows per core
    assert BP % NCORES == 0 and CAP % SBLK == 0 and SBLK % 128 == 0
    assert CAP % 128 == 0 and S % 16 == 0 and S % 128 == 0

    nc = bacc.Bacc("TRN2", target_bir_lowering=False, debug=False,
                   num_devices=NCORES)

    # ---- I/O -------------------------------------------------------------
    xT_hi = nc.dram_tensor("xT_hi", [F, B], BF16, kind="ExternalInput")
    xT_lo = nc.dram_tensor("xT_lo", [F, B], BF16, kind="ExternalInput")
    x_bf = nc.dram_tensor("x_bf", [B + 128, F], BF16, kind="ExternalInput")
    wgt_hi = nc.dram_tensor("wgt_hi", [128, FC * E], BF16, kind="ExternalInput")
    wgt_lo = nc.dram_tensor("wgt_lo", [128, FC * E], BF16, kind="ExternalInput")
    gb = nc.dram_tensor("gb", [1, E], FP32, kind="ExternalInput")
    W1 = nc.dram_tensor("W1", [EL, F, H], BF16, kind="ExternalInput")
    b1 = nc.dram_tensor("b1", [EL, H], BF16, kind="ExternalInput")
    W2 = nc.dram_tensor("W2", [EL, H, O], BF16, kind="ExternalInput")
    b2 = nc.dram_tensor("b2", [EL, O], BF16, kind="ExternalInput")
    out = nc.dram_tensor("out", [RSR, O], FP32, kind="ExternalOutput")
    if debug_outputs:
        dbg_inv = nc.dram_tensor("dbg_inv", [16, SW16], I16, kind="ExternalOutput")
        dbg_w = nc.dram_tensor("dbg_w", [128, SW128], FP32, kind="ExternalOutput")
        dbg_part = nc.dram_tensor("dbg_part", [128, O], partial_dtype,
                                  kind="ExternalOutput")
        dbg_sc = nc.dram_tensor("dbg_sc", [128, E], FP32, kind="ExternalOutput")
        dbg_wsb = nc.dram_tensor("dbg_wsb", [128, 8, EL], FP32,
                                 kind="ExternalOutput")


    # ---- internal DRAM ---------------------------------------------------
    partial = nc.dram_tensor("partial", [BP, O], partial_dtype, kind="Internal")
    # slot table: 8192 rows x 64 fp32 cols (256B rows for dma_scatter_add),
    # row = (s%128)*64 + s//128 (128-wrapped readback), col0 = token id,
    # col1 = gate weight. One scatter_add builds both.
    TROWS = 8192
    inv_f = nc.dram_tensor("inv_f", [TROWS, 64], FP32, kind="Internal")
    rs_out = nc.dram_tensor("rs_out", [RSR, O], partial_dtype, kind="Internal")

    with tile.TileContext(nc) as tc:
      for _it in range(n_iters):
        lp = tc.alloc_tile_pool(name="longlived", bufs=1)
        ewp = tc.alloc_tile_pool(name="expert_weights", bufs=1)
        mid = tc.alloc_tile_pool(name="midlived", bufs=1)

        # ---- phase 0: constants / prologue -------------------------------
        with tc.tile_critical():
            ids_all = mid.tile([128, NT, EL], I16)
            nc.gpsimd.iota(ids_all[:], base=0, channel_multiplier=1,
                           pattern=[[128, NT], [0, EL]])
            nc.gpsimd.load_library(library_config.mlp)

        ident = lp.tile([128, 128], FP32)
        nc.vector.memset(ident[:], 0.0)
        nc.gpsimd.affine_select(out=ident[:], in_=ident[:],
                                compare_op=mybir.AluOpType.not_equal, fill=1.0,
                                base=0, channel_multiplier=1, pattern=[[-1, 128]])
        ident_bf = lp.tile([128, 128], BF16)
        nc.vector.tensor_copy(out=ident_bf[:], in_=ident[:])
        # L[x,y] = 1 if x <= y (inclusive-cumsum matmul weight)
        L = mid.tile([128, 128], FP32)
        nc.vector.memset(L[:], 0.0)
        nc.gpsimd.affine_select(out=L[:], in_=L[:],
                                compare_op=mybir.AluOpType.is_gt, fill=1.0,
                                base=0, channel_multiplier=1, pattern=[[-1, 128]])
        ones_col = mid.tile([128, 1], FP32)
        nc.vector.memset(ones_col[:], 1.0)
        ones_row = mid.tile([1, 128], FP32)
        nc.vector.memset(ones_row[:], 1.0)
        ones_sblk_bf = lp.tile([1, SBLK], BF16)
        nc.vector.memset(ones_sblk_bf[:], 1.0)
        ones_chunk = mid.tile([1, CHUNK], FP32)
        nc.vector.memset(ones_chunk[:], 1.0)

        gb_sb = mid.tile([1, E], FP32)
        nc.gpsimd.dma_start(out=gb_sb[:], in_=gb[:, :])
        wg_hi = mid.tile([128, FC, E], BF16)
        nc.gpsimd.dma_start(out=wg_hi[:], in_=wgt_hi[:, :])
        wg_lo = mid.tile([128, FC, E], BF16)
        nc.gpsimd.dma_start(out=wg_lo[:], in_=wgt_lo[:, :])

        # expert-0 weights: loaded on the gpsimd queue so the DMA overlaps
        # the router phase. Subsequent experts reuse the same tiles (WAR
        # dependencies handled by the tile framework).
        w1sb = [None] * EL
        w2sb = [None] * EL
        b1row = [None] * EL
        b2row = [None] * EL

        def load_expert_weights(e):
            w1sb[e] = ewp.tile([128, FC, H], BF16, tag="w1sb", name=f"w1sb{e}")
            nc.gpsimd.dma_start(
                out=w1sb[e][:],
                in_=W1.ap()[e, :, :].rearrange("(a p) b -> p a b", p=128))
            w2sb[e] = ewp.tile([128, HC, O], BF16, tag="w2sb", name=f"w2sb{e}")
            nc.gpsimd.dma_start(
                out=w2sb[e][:],
                in_=W2.ap()[e, :, :].rearrange("(a p) b -> p a b", p=128))
            b1row[e] = ewp.tile([1, H], BF16, tag="b1row", name=f"b1row{e}")
            nc.gpsimd.dma_start(out=b1row[e][:], in_=b1[e:e + 1, :])
            b2row[e] = ewp.tile([1, O], BF16, tag="b2row", name=f"b2row{e}")
            nc.gpsimd.dma_start(out=b2row[e][:], in_=b2[e:e + 1, :])

        load_expert_weights(0)

        # zero-init partial (vector queue; overlaps router), prefill idx/w
        with tc.tile_pool(name="prolog", bufs=1) as prol:
            ZR = 8  # partial rows zeroed per partition per DMA
            zt = prol.tile([128, ZR, O], partial_dtype)
            nc.vector.memset(zt[:], 0.0)
            nz = BP // (128 * ZR)
            for r in range(nz):
                nc.scalar.dma_start(
                    out=partial.ap()[r * 128 * ZR:(r + 1) * 128 * ZR, :]
                        .rearrange("(a p) b -> p a b", p=128),
                    in_=zt[:])
            rem = BP - nz * 128 * ZR
            if rem:
                assert rem % 128 == 0
                nc.scalar.dma_start(
                    out=partial.ap()[nz * 128 * ZR:BP, :]
                        .rearrange("(a p) b -> p a b", p=128),
                    in_=zt[:, 0:rem // 128, :])
            TBR = TROWS // 128  # table rows per partition
            zf = prol.tile([128, TBR * 64], FP32)
            nc.vector.memset(zf[:], 0.0)
            nc.gpsimd.dma_start(
                out=inv_f.ap().rearrange("(p c) b -> p (c b)", p=128), in_=zf[:])

        # router state kept across phases
        w_sb = mid.tile([128, NT, EL], FP32)
        cums_sb = mid.tile([128, NT, EL], FP32)
        assign_sb = mid.tile([128, NT, EL], FP32)
        tpp = tc.alloc_tile_pool(name="tot_ps", bufs=1, space="PSUM")
        tot_ps = tpp.tile([EL, NT], FP32)

        # ---- phase 1: router ------------------------------------------
        with tc.tile_pool(name="router_sb", bufs=3) as rsb, \
             tc.tile_pool(name="router_ps", bufs=2, space="PSUM") as rps, \
             tc.tile_pool(name="router_ps2", bufs=2, space="PSUM") as rps2:
            for c in range(NCH):
                t0 = c * CHUNK
                xhi = rsb.tile([128, FC, CHUNK], BF16, tag="xhi")
                nc.sync.dma_start(
                    out=xhi[:],
                    in_=xT_hi.ap()[:, t0:t0 + CHUNK]
                        .rearrange("(a p) b -> p a b", p=128))
                xlo = rsb.tile([128, FC, CHUNK], BF16, tag="xlo")
                nc.sync.dma_start(
                    out=xlo[:],
                    in_=xT_lo.ap()[:, t0:t0 + CHUNK]
                        .rearrange("(a p) b -> p a b", p=128))
                # scoresT [E, CHUNK] = Wg_hi@x_hi + Wg_hi@x_lo + Wg_lo@x_hi + gb
                scp = rps.tile([E, CHUNK], FP32, tag="scp")
                for fc in range(FC):
                    nc.tensor.matmul(out=scp[:], lhsT=wg_hi[:, fc, :],
                                     rhs=xhi[:, fc, :],
                                     start=(fc == 0), stop=False)
                for fc in range(FC):
                    nc.tensor.matmul(out=scp[:], lhsT=wg_hi[:, fc, :],
                                     rhs=xlo[:, fc, :], start=False, stop=False)
                for fc in range(FC):
                    nc.tensor.matmul(out=scp[:], lhsT=wg_lo[:, fc, :],
                                     rhs=xhi[:, fc, :], start=False, stop=False)
                nc.tensor.matmul(out=scp[:], lhsT=gb_sb[:], rhs=ones_chunk[:],
                                 start=False, stop=True)
                scT = rsb.tile([E, CHUNK], FP32, tag="scT")
                nc.scalar.activation(out=scT[:], in_=scp[:],
                                     func=mybir.ActivationFunctionType.Copy)
                for j in range(TPC):
                    T = c * TPC + j
                    tps = rps2.tile([128, E], FP32, tag="tps")
                    nc.tensor.transpose(out=tps[:],
                                        in_=scT[:, j * 128:(j + 1) * 128],
                                        identity=ident[0:E, 0:E])
                    sc = rsb.tile([128, E], FP32, tag="sc")
                    nc.vector.tensor_copy(out=sc[:], in_=tps[:])
                    if debug_outputs and T == 0:
                        nc.gpsimd.dma_start(out=dbg_sc[:, :], in_=sc[:])
                    m8 = rsb.tile([128, 8], FP32, tag="m8")
                    nc.vector.max(out=m8[:], in_=sc[:])
                    nm1 = rsb.tile([128, 1], FP32, tag="nm1")
                    nc.vector.tensor_scalar_mul(nm1[:], m8[:, 0:1], -1.0)
                    # d = 1 + exp(m2 - m1); rd = 1/d
                    e2 = rsb.tile([128, 1], FP32, tag="e2")
                    nc.scalar.activation(out=e2[:], in_=m8[:, 1:2],
                                         func=mybir.ActivationFunctionType.Exp,
                                         bias=nm1[:, 0:1], scale=1.0)
                    d = rsb.tile([128, 1], FP32, tag="d")
                    nc.vector.tensor_scalar_add(d[:], e2[:], 1.0)
                    rd = rsb.tile([128, 1], FP32, tag="rd")
                    nc.vector.reciprocal(out=rd[:], in_=d[:])
                    # local-expert weights and assignment
                    el_ = rsb.tile([128, EL], FP32, tag="el_")
                    nc.scalar.activation(out=el_[:], in_=sc[:, 0:EL],
                                         func=mybir.ActivationFunctionType.Exp,
                                         bias=nm1[:, 0:1], scale=1.0)
                    wl = rsb.tile([128, EL], FP32, tag="wl")
                    nc.vector.tensor_scalar_mul(wl[:], el_[:], rd[:, 0:1])
                    al = rsb.tile([128, EL], FP32, tag="al")
                    nc.vector.tensor_scalar(out=al[:], in0=sc[:, 0:EL],
                                            scalar1=m8[:, 1:2], scalar2=None,
                                            op0=mybir.AluOpType.is_ge)
                    nc.vector.tensor_tensor(out=w_sb[:, T, :], in0=wl[:],
                                            in1=al[:], op=mybir.AluOpType.mult)
                    nc.vector.tensor_copy(out=assign_sb[:, T, :], in_=al[:])
                    cmp_ = rps2.tile([128, EL], FP32, tag="cmp_")
                    nc.tensor.matmul(out=cmp_[:], lhsT=L[:], rhs=al[:],
                                     start=True, stop=True)
                    nc.vector.tensor_copy(out=cums_sb[:, T, :], in_=cmp_[:])
                    nc.tensor.matmul(out=tot_ps[:, T:T + 1], lhsT=al[:],
                                     rhs=ones_col[:], start=True, stop=True)

        # ---- phase 2: capacity offsets ----------------------------------
        tot_sb = mid.tile([EL, NT], FP32)
        nc.vector.tensor_copy(out=tot_sb[:], in_=tot_ps[:])
        znt = mid.tile([EL, NT], FP32)
        nc.vector.memset(znt[:], 0.0)
        incl = mid.tile([EL, NT], FP32)
        nc.vector.tensor_tensor_scan(out=incl[:], data0=tot_sb[:], data1=znt[:],
                                     initial=0.0, op0=mybir.AluOpType.add,
                                     op1=mybir.AluOpType.add)
        excl = mid.tile([EL, NT], FP32)
        nc.vector.tensor_tensor(out=excl[:], in0=incl[:], in1=tot_sb[:],
                                op=mybir.AluOpType.subtract)
        tpp.release()

        # ---- phase 3: batched slot computation + 2 indirect scatters ----
        with tc.tile_pool(name="p3sb", bufs=1) as p3, \
             tc.tile_pool(name="p3ps", bufs=1, space="PSUM") as p3p:
            offb = p3.tile([128, NT, EL], FP32)
            for e in range(EL):
                orow = p3.tile([1, NT], FP32, tag=f"orow{e}", name=f"orow{e}")
                nc.gpsimd.dma_start(out=orow[:], in_=excl[e:e + 1, :])
                ofp = p3p.tile([128, NT], FP32, tag=f"ofp{e}", name=f"ofp{e}")
                nc.tensor.matmul(out=ofp[:], lhsT=ones_row[:],
                                 rhs=orow[:], start=True, stop=True)
                nc.vector.tensor_copy(out=offb[:, :, e], in_=ofp[:])
            gi = p3.tile([128, NT, EL], FP32)
            nc.vector.tensor_tensor(out=gi[:], in0=cums_sb[:], in1=offb[:],
                                    op=mybir.AluOpType.add)
            le = p3.tile([128, NT, EL], FP32)
            nc.vector.tensor_scalar(out=le[:], in0=gi[:], scalar1=float(CAP),
                                    scalar2=None, op0=mybir.AluOpType.is_le)
            kept = p3.tile([128, NT, EL], FP32)
            nc.vector.tensor_tensor(out=kept[:], in0=le[:], in1=assign_sb[:],
                                    op=mybir.AluOpType.mult)
            cbase = p3.tile([128, NT, EL], FP32)
            for e in range(EL):
                nc.vector.memset(cbase[:, :, e], float(e * CAP - 1))
            slotg = p3.tile([128, NT, EL], FP32)
            nc.vector.tensor_tensor(out=slotg[:], in0=gi[:], in1=cbase[:],
                                    op=mybir.AluOpType.add)
            kept8 = p3.tile([128, NT, EL], mybir.dt.uint8)
            nc.vector.tensor_copy(out=kept8[:], in_=kept[:])
            slotm = p3.tile([128, NT, EL], FP32)
            nc.vector.memset(slotm[:], float(S))  # dropped -> dump row S
            nc.vector.copy_predicated(out=slotm[:], mask=kept8[:], data=slotg[:])
            # table row permutations (computed in exact int32 arithmetic):
            #   inv row = (s & 15) << 9 | s >> 4 ; w row = (s & 127) << 6 | s >> 7
            s_i = p3.tile([128, NT, EL], I32)
            nc.vector.tensor_copy(out=s_i[:], in_=slotm[:])

            def rowperm(name, maskv, shl, shr):
                lo = p3.tile([128, NT, EL], I32, tag=f"{name}lo", name=f"{name}lo")
                nc.vector.tensor_scalar(out=lo[:], in0=s_i[:], scalar1=maskv,
                                        scalar2=None,
                                        op0=mybir.AluOpType.bitwise_and)
                nc.vector.tensor_scalar(out=lo[:], in0=lo[:], scalar1=shl,
                                        scalar2=None,
                                        op0=mybir.AluOpType.logical_shift_left)
                hi = p3.tile([128, NT, EL], I32, tag=f"{name}hi", name=f"{name}hi")
                nc.vector.tensor_scalar(out=hi[:], in0=s_i[:], scalar1=shr,
                                        scalar2=None,
                                        op0=mybir.AluOpType.logical_shift_right)
                nc.vector.tensor_tensor(out=lo[:], in0=lo[:], in1=hi[:],
                                        op=mybir.AluOpType.add)
                rf = p3.tile([128, NT, EL], FP32, tag=f"{name}f", name=f"{name}f")
                nc.vector.tensor_copy(out=rf[:], in_=lo[:])
                return rf

            rw_f = rowperm("rw", 127, 6, 7)    # table rows (128-wrap)
            # wrapped-16 pair-index table for dma_scatter_add: pair
            # i = g*128 + p (g = T*EL + e); idx[i] lives at wr[i%16, i//16]
            # -> wr[q, g, d] = rows[16*d + q, g]. Done as 8 PE selector
            # matmuls (exact fp32) + strided DVE convert-copies.
            GG = NT * EL
            wr_w = p3.tile([128, GG, 8], I16)
            for d_ in range(8):
                sps = p3p.tile([16, GG], FP32, tag="sps", name="sps")
                nc.tensor.matmul(out=sps[:],
                                 lhsT=ident[:, 16 * d_:16 * d_ + 16],
                                 rhs=rw_f[:].rearrange("p a b -> p (a b)"),
                                 start=True, stop=True)
                nc.vector.tensor_copy(out=wr_w[0:16, :, d_], in_=sps[:])
            nc.gpsimd.dma_start(out=wr_w[16:32, :, :], in_=wr_w[0:16, :, :])
            nc.gpsimd.dma_start(out=wr_w[32:64, :, :], in_=wr_w[0:32, :, :])
            nc.gpsimd.dma_start(out=wr_w[64:128, :, :], in_=wr_w[0:64, :, :])
            ids_f = p3.tile([128, NT, EL], FP32)
            nc.vector.tensor_copy(out=ids_f[:], in_=ids_all[:])
            NCHK = 8
            GPC = GG // NCHK          # pair-groups per chunk
            NIC = 128 * GPC           # indices per chunk
            with tc.tile_pool(name="p3db", bufs=2) as p3db:
                for ch in range(NCHK):
                    g0 = ch * GPC
                    t0_, t1_ = g0 // EL, (g0 + GPC) // EL
                    vin = p3db.tile([128, GPC, 64], FP32, tag="vin")
                    nc.vector.memset(vin[:], 0.0)
                    nc.vector.tensor_copy(out=vin[:, :, 0],
                                          in_=ids_f[:, t0_:t1_, :])
                    nc.vector.tensor_copy(out=vin[:, :, 1],
                                          in_=w_sb[:, t0_:t1_, :])
                    if not skip_dispatch_build:
                        nc.gpsimd.dma_scatter_add(
                            out_ap=inv_f[:, :], in_ap=vin[:],
                            idxs_ap=wr_w[:, g0:g0 + GPC, :],
                            num_idxs=NIC, num_idxs_reg=NIC, elem_size=64)
        if debug_outputs:
            nc.gpsimd.dma_start(out=dbg_wsb[:, :, :], in_=w_sb[:, 0:8, :])
        mid.release()

        # ---- phase 4: wrapped index tile + slot weights -----------------
        # One big contiguous table readback; col0/col1 extracted on-chip;
        # the 16-wrapped gather idx tile is rebuilt from the 128-wrapped
        # token ids with 8 PE selector matmuls (exact fp32).
        with tc.tile_pool(name="p4", bufs=1) as p4, \
             tc.tile_pool(name="p4ps", bufs=2, space="PSUM") as p4p:
            TBR = TROWS // 128
            invtab = p4.tile([128, TBR, 64], FP32)
            nc.gpsimd.dma_start(
                out=invtab[:],
                in_=inv_f.ap().rearrange("(p c) b -> p c b", p=128))
            w_sb128 = lp.tile([128, SW128], FP32)
            nc.vector.tensor_copy(out=w_sb128[:], in_=invtab[:, 0:SW128, 1])
            inv_v = p4.tile([128, SW128], FP32)
            nc.vector.tensor_copy(out=inv_v[:], in_=invtab[:, 0:SW128, 0])
            # idx_all[q, m*8+d] = inv_v[16d+q, m]
            idx_all = lp.tile([128, SW128, 8], I16)
            for d_ in range(8):
                ips = p4p.tile([16, SW128], FP32, tag="ips", name="ips")
                nc.tensor.matmul(out=ips[:],
                                 lhsT=ident[:, 16 * d_:16 * d_ + 16],
                                 rhs=inv_v[:], start=True, stop=True)
                nc.vector.tensor_copy(out=idx_all[0:16, :, d_], in_=ips[:])
            nc.gpsimd.dma_start(out=idx_all[16:32, :, :], in_=idx_all[0:16, :, :])
            nc.gpsimd.dma_start(out=idx_all[32:64, :, :], in_=idx_all[0:32, :, :])
            nc.gpsimd.dma_start(out=idx_all[64:128, :, :], in_=idx_all[0:64, :, :])
        if debug_outputs:
            nc.gpsimd.dma_start(out=dbg_inv[:, :], in_=idx_all[0:16, :, :])
            nc.gpsimd.dma_start(out=dbg_w[:, :], in_=w_sb128[:])

        # ---- phase 5: experts -------------------------------------------
        with tc.tile_pool(name="exp_sb", bufs=1) as esb, \
             tc.tile_pool(name="exp_db", bufs=2) as edb, \
             tc.tile_pool(name="exp_ps", bufs=2, space="PSUM") as eps, \
             tc.tile_pool(name="exp_tp", bufs=2, space="PSUM") as etp:
            for e in range(EL):
                if e > 0:
                    load_expert_weights(e)
                for blk in range(NBLK):
                    s0 = e * CAP + blk * SBLK
                    idxs = idx_all[:, s0 // 128:(s0 + SBLK) // 128, :]
                    bufT = edb.tile([128, FC, SBLK], BF16, tag="bufT")
                    nc.gpsimd.dma_gather(out_ap=bufT[:], in_ap=x_bf[:, :],
                                         idxs_ap=idxs, num_idxs=SBLK,
                                         num_idxs_reg=SBLK, elem_size=F,
                                         transpose=True)
                    hT = esb.tile([128, HC, SBLK], BF16, tag="hT")
                    for hc in range(HC):
                        ps = eps.tile([128, SBLK], FP32, tag="mmps")
                        for fc in range(FC):
                            nc.tensor.matmul(
                                out=ps[:],
                                lhsT=w1sb[e][:, fc, hc * 128:(hc + 1) * 128],
                                rhs=bufT[:, fc, :],
                                start=(fc == 0), stop=False)
                        nc.tensor.matmul(
                            out=ps[:],
                            lhsT=b1row[e][0:1, hc * 128:(hc + 1) * 128],
                            rhs=ones_sblk_bf[:], start=False, stop=True)
                        nc.scalar.activation(out=hT[:, hc, :], in_=ps[:],
                                             func=mybir.ActivationFunctionType.Relu)
                    yT = esb.tile([128, OC, SBLK], BF16, tag="yT")
                    for oc in range(OC):
                        ps2 = eps.tile([128, SBLK], FP32, tag="mmps")
                        for hc in range(HC):
                            nc.tensor.matmul(
                                out=ps2[:],
                                lhsT=w2sb[e][:, hc, oc * 128:(oc + 1) * 128],
                                rhs=hT[:, hc, :],
                                start=(hc == 0), stop=False)
                        nc.tensor.matmul(
                            out=ps2[:],
                            lhsT=b2row[e][0:1, oc * 128:(oc + 1) * 128],
                            rhs=ones_sblk_bf[:], start=False, stop=True)
                        nc.scalar.activation(out=yT[:, oc, :], in_=ps2[:],
                                             func=mybir.ActivationFunctionType.Copy)
                    # combine: transpose to slot-major, scale by gate weight
                    # (Activation engine), scatter-add into token rows.
                    ysc = esb.tile([128, G, O], partial_dtype, tag="ysc")
                    c0 = s0 // 128
                    for g_i in range(G):
                        wslg = w_sb128[:, c0 + g_i:c0 + g_i + 1]
                        for oc in range(OC):
                            tp = etp.tile([128, 128], BF16, tag="tp")
                            nc.tensor.transpose(
                                out=tp[:],
                                in_=yT[:, oc, g_i * 128:(g_i + 1) * 128],
                                identity=ident_bf[:])
                            nc.scalar.activation(
                                out=ysc[:, g_i, oc * 128:(oc + 1) * 128],
                                in_=tp[:],
                                func=mybir.ActivationFunctionType.Copy,
                                scale=wslg)
                    nc.gpsimd.dma_scatter_add(out_ap=partial[:, :], in_ap=ysc[:],
                                              idxs_ap=idxs, num_idxs=SBLK,
                                              num_idxs_reg=SBLK, elem_size=O)

        if debug_outputs:
            with tc.tile_pool(name="dbgp", bufs=1) as dbp:
                dpt = dbp.tile([128, O], partial_dtype)
                nc.gpsimd.dma_start(out=dpt[:], in_=partial[0:128, :])
                nc.gpsimd.dma_start(out=dbg_part[:, :], in_=dpt[:])

        # ---- phase 6: ReduceScatter + output ----------------------------
        if skip_collective:
            nc.gpsimd.dma_start(out=rs_out[:, :], in_=partial[0:RSR, :])
        else:
            nc.gpsimd.collective_compute(
                "ReduceScatter", mybir.AluOpType.add,
                replica_groups=[list(range(NCORES))],
                ins=[partial.ap().opt()], outs=[rs_out.ap().opt()])
        with tc.tile_pool(name="outp", bufs=2) as op_:
            for r in range(RSR // 128):
                ot = op_.tile([128, O], partial_dtype, tag="ot")
                nc.gpsimd.dma_start(out=ot[:], in_=rs_out[r * 128:(r + 1) * 128, :])
                if partial_dtype == FP32:
                    nc.sync.dma_start(out=out[r * 128:(r + 1) * 128, :], in_=ot[:])
                else:
                    of = op_.tile([128, O], FP32, tag="of")
                    nc.vector.tensor_copy(out=of[:], in_=ot[:])
                    nc.sync.dma_start(out=out[r * 128:(r + 1) * 128, :], in_=of[:])
            if RSR % 128:
                r0 = (RSR // 128) * 128
                rem = RSR - r0
                ot = op_.tile([128, O], partial_dtype, tag="ot")
                nc.gpsimd.dma_start(out=ot[0:rem, :], in_=rs_out[r0:RSR, :])
                if partial_dtype == FP32:
                    nc.sync.dma_start(out=out[r0:RSR, :], in_=ot[0:rem, :])
                else:
                    of = op_.tile([128, O], FP32, tag="of")
                    nc.vector.tensor_copy(out=of[0:rem, :], in_=ot[0:rem, :])
                    nc.sync.dma_start(out=out[r0:RSR, :], in_=of[0:rem, :])

        ewp.release()
        lp.release()

    nc.compile()
    return nc


def make_in_maps(inputs, E=16, NCORES=8):
    """Host-side prep: bf16 casts, feature transpose + hi/lo split, gate
    weight permute/swizzle, expert weight slicing. Returns per-core dicts."""
    import ml_dtypes
    bf = ml_dtypes.bfloat16
    EL = E // NCORES
    f = np.asarray(inputs["features"], dtype=np.float32)
    B, F = f.shape
    FC = F // 128
    f_hi32 = f.astype(bf).astype(np.float32)
    xT_hi = np.ascontiguousarray(f_hi32.astype(bf).T)
    xT_lo = np.ascontiguousarray((f - f_hi32).astype(bf).T)
    x_bf = np.zeros((B + 128, F), dtype=bf)
    x_bf[:B] = f.astype(bf)

    Wg = np.asarray(inputs["Wg"], dtype=np.float32)
    gb_full = (np.asarray(inputs["bg"], dtype=np.float32)
               + np.asarray(inputs["expert_bias"], dtype=np.float32))
    W1 = np.asarray(inputs["W1"], dtype=np.float32).astype(bf)
    b1 = np.asarray(inputs["b1"], dtype=np.float32).astype(bf)
    W2 = np.asarray(inputs["W2"], dtype=np.float32).astype(bf)
    b2 = np.asarray(inputs["b2"], dtype=np.float32).astype(bf)

    def swizzle_wgt(Wp):
        # [E, F] -> [128, FC*E]: out[p, fc*E+e] = Wp[e, fc*128+p]
        t = Wp.T.reshape(FC, 128, E).transpose(1, 0, 2).reshape(128, FC * E)
        return np.ascontiguousarray(t)

    in_maps = []
    for i in range(NCORES):
        mine = list(range(i * EL, (i + 1) * EL))
        rest = [e for e in range(E) if e not in mine]
        perm = mine + rest
        Wgp = Wg[perm]
        Wh32 = Wgp.astype(bf).astype(np.float32)
        wgt_hi = swizzle_wgt(Wh32.astype(bf))
        wgt_lo = swizzle_wgt((Wgp - Wh32).astype(bf))
        in_maps.append({
            "xT_hi": xT_hi,
            "xT_lo": xT_lo,
            "x_bf": x_bf,
            "wgt_hi": wgt_hi,
            "wgt_lo": wgt_lo,
            "gb": np.ascontiguousarray(gb_full[perm].reshape(1, E)),
            "W1": W1[i * EL:(i + 1) * EL],
            "b1": b1[i * EL:(i + 1) * EL],
            "W2": W2[i * EL:(i + 1) * EL],
            "b2": b2[i * EL:(i + 1) * EL],
        })
    return in_maps


_NC_CACHE = {}


def kernel(**inputs):
    from concourse.bass_utils import run_bass_kernel_spmd
    B, F = 16384, 1024
    H, O, E, NCORES = 4096, 1024, 16, 8
    key = "full"
    if key not in _NC_CACHE:
        _NC_CACHE[key] = build_moe(B=B, F=F, H=H, O=O, E=E, NCORES=NCORES,
                                   partial_dtype=BF16)
    nc = _NC_CACHE[key]
    in_maps = make_in_maps(inputs, E=E, NCORES=NCORES)
    res = run_bass_kernel_spmd(nc, in_maps, core_ids=list(range(NCORES)))
    shards = [res.results[i]["out"] for i in range(NCORES)]
    full = np.concatenate(shards, axis=0)[:B]
    return full.astype(np.float32)


if __name__ == "__main__":
    data = np.load("/root/problem/work/ref_data.npz")
    inputs = {k: data[k] for k in
              ["features", "Wg", "bg", "W1", "b1", "W2", "b2", "expert_bias"]}
    outp = kernel(**inputs)
    exp = data["expected"]
    err = np.linalg.norm(outp - exp) / np.linalg.norm(exp)
    print("Relative error:", err)

